# revision 18
# baseline (speedup 1.0000x reference)
"""Trainium2 Bass kernel for nn_Curating_of_attention_loss.

loss = sum(sattn * pen(batch)) where pen is a per-sample [32,32,32,32] penalty
built from a 2x2-grid Gram-distance matrix pf [256,256] upsampled 2x in all
four spatial dims.  Because pen[s,y,x,y2,x2] = pf'[s, g(y,x), g(y2,x2)], the
loss factorizes as

    loss = sum_s sum_{p,q} PF_eff[s,p,q] * S[s,p,q]

with S the 16:1 (4 i-pixels x 4 j-pixels) pooled sattn.  Reading the 256 MB
sattn once and pooling it is the memory-bound bulk; everything else is tiny.

Per-core layout (pure data parallel over batch, 8 samples/core):
  Phase A: channel-pair products via broadcast DMAs, pooled to Gram matrices
           G[(s,ij), grid] with one strided XY-reduce; the unscaled distance
           D = -2*G^T G + n2[q] + n2[p] is built with three accumulating
           K=128 matmuls (zero-padding via per-partition block masks; the
           n2[q] row-broadcast is an all-ones matmul; the n2[p] column is a
           diag(n2) matmul, diag built as identity*n2 on VectorE);
           pf = D/rowmax + 1 (pf is scale-invariant in D so all constant
           scalings are dropped, and the reference's exact row-min of 0 is a
           no-op); the reference's sequential-replacement aliasing quirk is
           reproduced with a (D[:,1]==rowmax) predicated copy.
  Phase B: sattn i-pooling via TensorE matmuls against constant 0/1 pooling
           weights (float32r streams at full rate; 0/1 weights are exact),
           j-pooling via one strided XY-reduce per PSUM tile, then a fused
           multiply-reduce dot against PF chained over all (sample, chunk)
           pairs into a [128,1] accumulator.

Host side shards inputs batch-wise across the 8 cores and sums the per-core
[128,1] partials (the all-reduce of the sharding hint).
"""

import sys

if "/opt/trn_rl_repo" not in sys.path:
    sys.path.insert(0, "/opt/trn_rl_repo")

import numpy as np

N_CORES = 8
SPC = 8  # samples per core (64 total / 8 cores)

# Phase-B matmul dtype: float32r streams at 1 cycle/row (vs 4 for float32).
USE_F32R = True
# debug bisect knobs: emit only part of the kernel
EMIT_PHASE_A = True
EMIT_PHASE_B = True


def _pool_weights() -> np.ndarray:
    """4 shifted pooling matrices [128 pixels x 128 grid-slots].

    For i-chunk cc within a group of 4 (512 pixel rows), pixel k of the chunk
    maps to grid slot 32*cc + 16*(k//64) + (k%32)//2; the four accumulating
    matmuls write disjoint 32-row bands of one [128, 512] PSUM tile.
    """
    pmw = np.zeros((4, 128, 128), np.float32)
    k = np.arange(128)
    m = 16 * (k // 64) + (k % 32) // 2
    for cc in range(4):
        pmw[cc, k, 32 * cc + m] = 1.0
    return pmw


def _block_masks() -> np.ndarray:
    """bmask[s4, k] = 1 where partition k belongs to sample-slot s4's 9 rows."""
    bm = np.zeros((4, 128), np.float32)
    for s4 in range(4):
        bm[s4, 32 * s4:32 * s4 + 9] = 1.0
    return bm


def _emit(tc, batch, sattn, pmw_d, ident_d, bmask_d, out):
    from concourse import mybir
    from contextlib import ExitStack

    nc = tc.nc
    ALU = mybir.AluOpType
    AX = mybir.AxisListType
    F32 = mybir.dt.float32
    F32R = mybir.dt.float32r

    batch_v = batch.rearrange("s c h w -> s c (h w)")        # [8, 3, 1024]
    sattn_v = sattn.rearrange("s a b c d -> s (a b) (c d)")  # [8, 1024, 1024]

    with ExitStack() as top:
        cpool = top.enter_context(tc.tile_pool(name="consts", bufs=1))
        ones = cpool.tile([128, 256], F32)
        nc.vector.memset(ones[:], 1.0)
        ident = cpool.tile([128, 128], F32)
        nc.scalar.dma_start(out=ident[:], in_=ident_d)
        bm = cpool.tile([128, 4], F32)
        nc.scalar.dma_start(out=bm[:], in_=bmask_d.rearrange("a p -> p a"))
        BDT = F32R if USE_F32R else F32
        pw = cpool.tile([128, 4, 128], BDT)
        if USE_F32R:
            # f32r operands must be produced as f32r; SWDGE casts in-flight
            nc.gpsimd.dma_start(out=pw[:], in_=pmw_d.rearrange("c k m -> k c m"))
        else:
            nc.scalar.dma_start(out=pw[:], in_=pmw_d.rearrange("c k m -> k c m"))

        gpool = top.enter_context(tc.tile_pool(name="gram", bufs=2))
        mpool = top.enter_context(tc.tile_pool(name="masked", bufs=3))
        vpool = top.enter_context(tc.tile_pool(name="vecs", bufs=4))
        pfpool = top.enter_context(tc.tile_pool(name="pf", bufs=16))

        # ---------------- Phase A: pf matrices ----------------
        pf_tiles = {}
        if not EMIT_PHASE_A:
            for s in range(SPC):
                for pc in range(2):
                    pf = pfpool.tile([128, 256], F32, tag="pf",
                                     name=f"pfd{s}_{pc}")
                    nc.vector.memset(pf[:], 1.0)
                    pf_tiles[(s, pc)] = pf
        else:
            aps_ctx = tc.tile_pool(name="apsum", bufs=2, space="PSUM")
            nps_ctx = tc.tile_pool(name="npsum", bufs=2, space="PSUM")
            psA = aps_ctx.__enter__()
            psN = nps_ctx.__enter__()
            g_tiles = []
            for t in range(2):
                # Ba holds channel i replicated over j; Bb channel j; partition
                # 32*s4 + 3*i + j.  Unused partitions zeroed once.
                ba = gpool.tile([128, 1024], F32, tag="ba")
                bb = gpool.tile([128, 1024], F32, tag="bb")
                nc.vector.memset(ba[:], 0.0)
                nc.vector.memset(bb[:], 0.0)
                for s4 in range(4):
                    s = 4 * t + s4
                    b = 32 * s4
                    nc.gpsimd.dma_start(
                        out=ba[b:b + 9, :],
                        in_=batch_v[s].unsqueeze(1).broadcast_to((3, 3, 1024)))
                    nc.gpsimd.dma_start(
                        out=bb[b:b + 9, :],
                        in_=batch_v[s].unsqueeze(0).broadcast_to((3, 3, 1024)))
                # H[(s,ij), pix] = f_i * f_j, then pool 2x2 pixel blocks:
                # pix = 64a + 32t + 2b + u -> grid (a,b), parities (t,u)
                h = gpool.tile([128, 1024], F32, tag="h")
                nc.vector.tensor_mul(h[:], ba[:], bb[:])
                hv = h[:].rearrange("p (a t b u) -> p a b t u", a=16, t=2,
                                    b=16, u=2)
                g = gpool.tile([128, 256], F32, tag="g")
                gv = g[:].rearrange("p (a b) -> p a b", a=16, b=16)
                nc.vector.tensor_reduce(gv, hv, axis=AX.XY, op=ALU.add)
                gsq = gpool.tile([128, 256], F32, tag="gsq")
                nc.vector.tensor_mul(gsq[:], g[:], g[:])
                gm2 = gpool.tile([128, 256], F32, tag="gm2")
                nc.scalar.mul(gm2[:], g[:], -2.0)
                g_tiles.append((g, gsq, gm2))

            for s in range(SPC):
                t, s4 = divmod(s, 4)
                g, gsq, gm2 = g_tiles[t]
                # mask the lhsT operands down to this sample's 9 rows so the
                # K=128 contraction only sees its block
                gm2_s = mpool.tile([128, 256], F32, tag="gm2s")
                nc.vector.tensor_scalar_mul(gm2_s[:], gm2[:], bm[:, s4:s4 + 1])
                gsq_s = mpool.tile([128, 256], F32, tag="gsqs")
                nc.vector.tensor_scalar_mul(gsq_s[:], gsq[:], bm[:, s4:s4 + 1])
                # n2[q] broadcast to all rows: ones^T @ gsq_s
                p_n2 = psN.tile([128, 256], F32, tag="n2")
                nc.tensor.matmul(p_n2[:], lhsT=ones[:, 0:128], rhs=gsq_s[:, :],
                                 start=True, stop=True)
                for pc in range(2):
                    # diag(n2_chunk) = identity * n2 rows
                    dg = mpool.tile([128, 128], F32, tag="dg")
                    nc.vector.tensor_mul(dg[:], ident[:],
                                         p_n2[:, 128 * pc:128 * pc + 128])
                    # D = -2*G_s^T G + n2[q] + n2[p]  (three K=128 matmuls)
                    p_d = psA.tile([128, 256], F32, tag="d")
                    nc.tensor.matmul(p_d[:],
                                     lhsT=gm2_s[:, 128 * pc:128 * pc + 128],
                                     rhs=g[:, :], start=True, stop=False)
                    nc.tensor.matmul(p_d[:], lhsT=ones[:, 0:128],
                                     rhs=gsq_s[:, :], start=False, stop=False)
                    nc.tensor.matmul(p_d[:], lhsT=dg[:, :], rhs=ones[:, :],
                                     start=False, stop=True)
                    rmax = vpool.tile([128, 1], F32, tag="rmax")
                    nc.vector.tensor_reduce(rmax[:], p_d[:], axis=AX.X,
                                            op=ALU.max)
                    rinv = vpool.tile([128, 1], F32, tag="rinv")
                    nc.vector.reciprocal(rinv[:], rmax[:])
                    pf = pfpool.tile([128, 256], F32, tag="pf")
                    nc.vector.tensor_scalar(pf[:], p_d[:], scalar1=rinv[:],
                                            scalar2=1.0, op0=ALU.mult,
                                            op1=ALU.add)
                    # aliasing quirk: where D[:,1] attains the row max the
                    # reference overwrites grid-1 columns with pf[:,2]
                    mask = vpool.tile([128, 1], mybir.dt.int32, tag="mask")
                    nc.vector.tensor_tensor(mask[:], p_d[:, 1:2], rmax[:],
                                            op=ALU.is_equal)
                    nc.vector.copy_predicated(pf[:, 1:2], mask[:], pf[:, 2:3])
                    pf_tiles[(s, pc)] = pf
            nps_ctx.__exit__(None, None, None)
            aps_ctx.__exit__(None, None, None)

        # ---------------- Phase B: sattn pooling + dot ----------------
        bpool = top.enter_context(tc.tile_pool(name="sattn", bufs=4))
        spool = top.enter_context(tc.tile_pool(name="stile", bufs=3))
        dpool = top.enter_context(tc.tile_pool(name="dot", bufs=4))
        scpool = top.enter_context(tc.tile_pool(name="scratch", bufs=1))
        psB = top.enter_context(tc.tile_pool(name="bpsum", bufs=8,
                                             space="PSUM"))

        scr = scpool.tile([128, 256], F32)
        acc = None
        if not EMIT_PHASE_B:
            for s in range(SPC):
                for pc in range(2):
                    accn = dpool.tile([128, 1], F32, tag="acc",
                                      name=f"acc{s}_{pc}")
                    nc.vector.tensor_tensor_reduce(
                        out=scr[:], in0=pf_tiles[(s, pc)][:],
                        in1=ones[:, 0:256], scale=1.0,
                        scalar=(0.0 if acc is None else acc[:]),
                        op0=ALU.mult, op1=ALU.add, accum_out=accn[:])
                    acc = accn
        for s in (range(SPC) if EMIT_PHASE_B else []):
            for pc in range(2):
                at = bpool.tile([128, 4, 1024], BDT, tag="at")
                src = sattn_v[s, 512 * pc:512 * (pc + 1), :].rearrange(
                    "(c p) j -> p c j", p=128)
                if USE_F32R:
                    nc.gpsimd.dma_start(out=at[:], in_=src)
                else:
                    nc.sync.dma_start(out=at[:], in_=src)
                ps_tiles = [psB.tile([128, 512], F32, tag="ps", name=f"ps{jh}")
                            for jh in range(2)]
                for cc in range(4):
                    for jh in range(2):
                        nc.tensor.matmul(ps_tiles[jh][:], lhsT=pw[:, cc, :],
                                         rhs=at[:, cc, 512 * jh:512 * (jh + 1)],
                                         start=(cc == 0), stop=(cc == 3))
                stile = spool.tile([128, 256], F32, tag="s")
                for jh in range(2):
                    # j = 64a + 32t + 2b + u with (a,b) the grid coords and
                    # (t,u) the intra-grid parities: one XY-reduce pools 4:1
                    psv = ps_tiles[jh][:].rearrange("p (a t b u) -> p a b t u",
                                                    a=8, t=2, b=16, u=2)
                    sv = stile[:, 128 * jh:128 * (jh + 1)].rearrange(
                        "p (a b) -> p a b", a=8, b=16)
                    nc.vector.tensor_reduce(sv, psv, axis=AX.XY, op=ALU.add)
                accn = dpool.tile([128, 1], F32, tag="acc")
                nc.vector.tensor_tensor_reduce(
                    out=scr[:], in0=stile[:], in1=pf_tiles[(s, pc)][:],
                    scale=1.0, scalar=(0.0 if acc is None else acc[:]),
                    op0=ALU.mult, op1=ALU.add, accum_out=accn[:])
                acc = accn

        # per-core partial: [128,1]; the host sums partitions and cores
        nc.sync.dma_start(out=out, in_=acc[:, :])


def build_nc():
    import concourse.tile as tile
    from concourse import bacc, mybir

    F32 = mybir.dt.float32
    nc = bacc.Bacc("TRN2", target_bir_lowering=False, debug=False)
    batch = nc.dram_tensor("batch", [SPC, 3, 32, 32], F32,
                           kind="ExternalInput").ap()
    sattn = nc.dram_tensor("sattn", [SPC, 32, 32, 32, 32], F32,
                           kind="ExternalInput").ap()
    pmw = nc.dram_tensor("pmw", [4, 128, 128], F32, kind="ExternalInput").ap()
    ident = nc.dram_tensor("ident", [128, 128], F32,
                           kind="ExternalInput").ap()
    bmask = nc.dram_tensor("bmask", [4, 128], F32, kind="ExternalInput").ap()
    out = nc.dram_tensor("out", [128, 1], F32, kind="ExternalOutput").ap()
    with tile.TileContext(nc) as tc:
        _emit(tc, batch, sattn, pmw, ident, bmask, out)
    nc.compile()
    return nc


def make_in_maps(batch: np.ndarray, sattn: np.ndarray):
    pmw = _pool_weights()
    ident = np.eye(128, dtype=np.float32)
    bmask = _block_masks()
    in_maps = []
    for c in range(N_CORES):
        in_maps.append({
            "batch": np.ascontiguousarray(batch[SPC * c:SPC * (c + 1)],
                                          dtype=np.float32),
            "sattn": np.ascontiguousarray(sattn[SPC * c:SPC * (c + 1)],
                                          dtype=np.float32),
            "pmw": pmw,
            "ident": ident,
            "bmask": bmask,
        })
    return in_maps


_NC_CACHE = []


def kernel(batch: np.ndarray, sattn: np.ndarray) -> np.ndarray:
    from concourse.bass_utils import run_bass_kernel_spmd

    assert batch.shape == (64, 3, 32, 32) and sattn.shape == (64, 32, 32, 32, 32)
    if not _NC_CACHE:
        _NC_CACHE.append(build_nc())
    nc = _NC_CACHE[0]
    in_maps = make_in_maps(batch, sattn)
    res = run_bass_kernel_spmd(nc, in_maps, list(range(N_CORES)))
    partials = np.stack([res.results[c]["out"][:, 0] for c in range(N_CORES)])
    return np.float32(partials.sum(dtype=np.float32))
